# revision 1
# baseline (speedup 1.0000x reference)
"""Trainium2 Bass kernel for nn_Linear_28879360098368 (dense_mlp).

Computes y = x @ dequant(weight, scale).T where dequant multiplies each
128x128 block of weight by a scalar from `scale`.

Sharding (hardcoded): tensor-parallel over out_features — each of the 8
cores gets 12288/8 = 1536 output features (weight rows + matching scale
rows); x is replicated. No collectives: each core computes its y column
shard and the host concatenates.

Per-core device kernel: M=8192, K=4096, N=1536 bf16 matmul with fp32
accumulation. The weight shard (as wT = w.T, [K, N] bf16) is DMA'd into
SBUF once, dequantized in-place on VectorE (per-128-block scale
broadcast), and stays resident. x arrives as xT = x.T ([K, M] bf16) and
streams through SBUF in M-slabs of 512. TensorE accumulates over the
full K=4096 per PSUM tile (32 matmuls of k=128, n_free=512).

Host prep is layout-only: bf16 cast + transpose + shard slicing. All
dequant multiplies and matmul FLOPs run on device.
"""

from contextlib import ExitStack

import ml_dtypes
import numpy as np

import concourse.bacc as bacc
import concourse.mybir as mybir
import concourse.tile as tile
from concourse.bass_utils import run_bass_kernel_spmd

BF16 = ml_dtypes.bfloat16

# Problem shapes (hardcoded per contract).
B, S, IN, OUT = 4, 2048, 4096, 12288
NCORES = 8
M = B * S               # 8192 rows
K = IN                  # 4096 contraction
N = OUT // NCORES       # 1536 out-features per core
KB = K // 128           # 32 k-blocks
NB = N // 128           # 12 n-blocks per core
M_TILE = 512
M_SUB = M_TILE // 128   # 4
M_TILES = M // M_TILE   # 16
N_FREE = 512            # PSUM bank width (fp32)
N_CH = N // N_FREE      # 3

_nc_cache = []


def _build_nc():
    """Build (and cache) the per-core Bass program. Same program runs SPMD
    on all 8 cores; only the input data differs."""
    if _nc_cache:
        return _nc_cache[0]

    nc = bacc.Bacc("TRN2", target_bir_lowering=False, debug=False)
    xT = nc.dram_tensor("xT", [K, M], mybir.dt.bfloat16, kind="ExternalInput")
    wT = nc.dram_tensor("wT", [K, N], mybir.dt.bfloat16, kind="ExternalInput")
    # sc[p, kb, jb] = scale[jb, kb] replicated over the 128 partitions.
    sc = nc.dram_tensor("sc", [128, KB, NB], mybir.dt.float32, kind="ExternalInput")
    y = nc.dram_tensor("y", [M, N], mybir.dt.float32, kind="ExternalOutput")

    xT3 = xT.ap().rearrange("(ko p) m -> p ko m", p=128)   # [128, KB, M]
    wT3 = wT.ap().rearrange("(ko p) n -> p ko n", p=128)   # [128, KB, N]
    y3 = y.ap().rearrange("(mo p) n -> p mo n", p=128)     # [128, M//128, N]

    with tile.TileContext(nc) as tc, ExitStack() as ctx:
        wpool = ctx.enter_context(tc.tile_pool(name="wpool", bufs=1))
        cpool = ctx.enter_context(tc.tile_pool(name="cpool", bufs=1))
        xpool = ctx.enter_context(tc.tile_pool(name="xpool", bufs=2))
        opool = ctx.enter_context(tc.tile_pool(name="opool", bufs=4))
        ppool = ctx.enter_context(tc.tile_pool(name="ppool", bufs=8, space="PSUM"))

        scb = cpool.tile([128, KB, NB], mybir.dt.float32)
        nc.sync.dma_start(scb[:], sc.ap())

        # Resident weight shard: load + dequantize in place, one k-block
        # (= one [128, N] stripe) at a time so dequant pipelines with DMA.
        wsb = wpool.tile([128, KB, N], mybir.dt.bfloat16)
        for kb in range(KB):
            nc.sync.dma_start(wsb[:, kb], wT3[:, kb])
            w3 = wsb[:, kb].rearrange("p (j i) -> p j i", i=128)
            nc.vector.tensor_tensor(
                w3,
                w3,
                scb[:, kb, :, None].to_broadcast([128, NB, 128]),
                mybir.AluOpType.mult,
            )

        for mo in range(M_TILES):
            xsb = xpool.tile([128, KB, M_TILE], mybir.dt.bfloat16)
            # Split the 4MB slab load in two so the second half can land
            # while matmuls on the first half's k-blocks already run.
            half = KB // 2
            nc.sync.dma_start(xsb[:, :half], xT3[:, :half, ts(mo)])
            nc.sync.dma_start(xsb[:, half:], xT3[:, half:, ts(mo)])

            for ni in range(N_CH):
                for ms in range(M_SUB):
                    pt = ppool.tile([128, N_FREE], mybir.dt.float32, name="pt")
                    for kb in range(KB):
                        nc.tensor.matmul(
                            pt[:],
                            xsb[:, kb, ms * 128:(ms + 1) * 128],
                            wsb[:, kb, ni * N_FREE:(ni + 1) * N_FREE],
                            start=(kb == 0),
                            stop=(kb == KB - 1),
                        )
                    ot = opool.tile([128, N_FREE], mybir.dt.float32, name="ot")
                    nc.any.tensor_copy(ot[:], pt[:])
                    nc.sync.dma_start(
                        y3[:, mo * M_SUB + ms, ni * N_FREE:(ni + 1) * N_FREE],
                        ot[:],
                    )

    nc.compile()
    _nc_cache.append(nc)
    return nc


def ts(mo):
    return slice(mo * M_TILE, (mo + 1) * M_TILE)


def _prep_inputs(x, weight, scale):
    """Host-side layout prep + sharding. Returns per-core in_maps."""
    xT = np.ascontiguousarray(
        x.reshape(M, K).astype(BF16).T
    )  # [K, M] bf16, replicated to all cores
    in_maps = []
    for c in range(NCORES):
        w_c = weight[c * N:(c + 1) * N, :]           # [N, K] f32
        wT_c = np.ascontiguousarray(w_c.astype(BF16).T)  # [K, N] bf16
        s_c = scale[c * NB:(c + 1) * NB, :]          # [NB, KB] f32
        sc_c = np.ascontiguousarray(
            np.broadcast_to(s_c.T[None, :, :], (128, KB, NB))
        ).astype(np.float32)                         # [128, KB, NB]
        in_maps.append({"xT": xT, "wT": wT_c, "sc": sc_c})
    return in_maps


def run(x, weight, scale, **spmd_kwargs):
    """Build, run on 8 cores, gather. Returns (y_full, BassKernelResults)."""
    nc = _build_nc()
    in_maps = _prep_inputs(x, weight, scale)
    res = run_bass_kernel_spmd(nc, in_maps, core_ids=list(range(NCORES)), **spmd_kwargs)
    y = np.concatenate([r["y"] for r in res.results], axis=1)  # [M, OUT]
    return y.reshape(B, S, OUT).astype(np.float32), res


def kernel(x, weight, scale):
    y, _ = run(np.asarray(x), np.asarray(weight), np.asarray(scale))
    return y


# revision 2
# speedup vs baseline: 1.0198x; 1.0198x over previous
"""Trainium2 Bass kernel for nn_Linear_28879360098368 (dense_mlp).

Computes y = x @ dequant(weight, scale).T where dequant multiplies each
128x128 block of weight by a scalar from `scale`.

Sharding (hardcoded): tensor-parallel over out_features — each of the 8
cores gets 12288/8 = 1536 output features (weight rows + matching scale
rows); x is replicated. No collectives: each core computes its y column
shard and the host concatenates.

Per-core device kernel: M=8192, K=4096, N=1536 bf16 matmul with fp32
accumulation. The weight shard (as wT = w.T, [K, N] bf16) is DMA'd into
SBUF once, dequantized in-place on VectorE (per-128-block scale
broadcast), and stays resident. x arrives as xT = x.T ([K, M] bf16) and
streams through SBUF in M-slabs of 512. TensorE accumulates over the
full K=4096 per PSUM tile (32 matmuls of k=128, n_free=512).

Startup choreography: the first x slab loads on the Sync HWDGE ring
before anything else while the weight stripes load on the Scalar HWDGE
ring; slab 0's matmuls run kb-major across 8 concurrent PSUM chains so
TensorE consumes each k-block as VectorE finishes dequantizing it.

Host prep is layout-only: bf16 cast + transpose + shard slicing. All
dequant multiplies and matmul FLOPs run on device.
"""

from contextlib import ExitStack

import ml_dtypes
import numpy as np

import concourse.bacc as bacc
import concourse.mybir as mybir
import concourse.tile as tile
from concourse.bass_utils import run_bass_kernel_spmd

BF16 = ml_dtypes.bfloat16

# Problem shapes (hardcoded per contract).
B, S, IN, OUT = 4, 2048, 4096, 12288
NCORES = 8
M = B * S               # 8192 rows
K = IN                  # 4096 contraction
N = OUT // NCORES       # 1536 out-features per core
KB = K // 128           # 32 k-blocks
NB = N // 128           # 12 n-blocks per core
M_TILE = 512
M_SUB = M_TILE // 128   # 4
M_TILES = M // M_TILE   # 16
N_FREE = 512            # PSUM bank width (fp32)
N_CH = N // N_FREE      # 3

_nc_cache = []


def _mslice(mo):
    return slice(mo * M_TILE, (mo + 1) * M_TILE)


def _build_nc():
    """Build (and cache) the per-core Bass program. Same program runs SPMD
    on all 8 cores; only the input data differs."""
    if _nc_cache:
        return _nc_cache[0]

    nc = bacc.Bacc("TRN2", target_bir_lowering=False, debug=False)
    xT = nc.dram_tensor("xT", [K, M], mybir.dt.bfloat16, kind="ExternalInput")
    wT = nc.dram_tensor("wT", [K, N], mybir.dt.bfloat16, kind="ExternalInput")
    # sc[p, kb, jb] = scale[jb, kb] replicated over the 128 partitions.
    sc = nc.dram_tensor("sc", [128, KB, NB], mybir.dt.float32, kind="ExternalInput")
    y = nc.dram_tensor("y", [M, N], mybir.dt.float32, kind="ExternalOutput")

    xT3 = xT.ap().rearrange("(ko p) m -> p ko m", p=128)   # [128, KB, M]
    wT3 = wT.ap().rearrange("(ko p) n -> p ko n", p=128)   # [128, KB, N]
    y3 = y.ap().rearrange("(mo p) n -> p mo n", p=128)     # [128, M//128, N]

    with tile.TileContext(nc) as tc, ExitStack() as ctx:
        wpool = ctx.enter_context(tc.tile_pool(name="wpool", bufs=1))
        cpool = ctx.enter_context(tc.tile_pool(name="cpool", bufs=1))
        xpool = ctx.enter_context(tc.tile_pool(name="xpool", bufs=2))
        opool = ctx.enter_context(tc.tile_pool(name="opool", bufs=6))
        ppool = ctx.enter_context(tc.tile_pool(name="ppool", bufs=8, space="PSUM"))

        scb = cpool.tile([128, KB, NB], mybir.dt.float32)
        nc.sync.dma_start(scb[:], sc.ap())

        # Slab 0 of x loads first (Sync ring), in quarters so the early
        # k-blocks land before the weight stripes finish.
        xsb0 = xpool.tile([128, KB, M_TILE], mybir.dt.bfloat16, name="xsb")
        q = KB // 4
        for i in range(4):
            nc.sync.dma_start(xsb0[:, i * q:(i + 1) * q], xT3[:, i * q:(i + 1) * q, _mslice(0)])

        # Resident weight shard on the Scalar HWDGE ring (keeps the Sync
        # ring free for x/y traffic): load + dequantize one k-block
        # (= one [128, N] stripe) at a time so dequant pipelines with DMA.
        wsb = wpool.tile([128, KB, N], mybir.dt.bfloat16)
        for kb in range(KB):
            nc.scalar.dma_start(wsb[:, kb], wT3[:, kb])
            w3 = wsb[:, kb].rearrange("p (j i) -> p j i", i=128)
            nc.vector.tensor_tensor(
                w3,
                w3,
                scb[:, kb, :, None].to_broadcast([128, NB, 128]),
                mybir.AluOpType.mult,
            )

        def evict(pt, mo, ms, ni):
            ot = opool.tile([128, N_FREE], mybir.dt.float32, name="ot")
            nc.any.tensor_copy(ot[:], pt[:])
            nc.sync.dma_start(
                y3[:, mo * M_SUB + ms, ni * N_FREE:(ni + 1) * N_FREE], ot[:]
            )

        chains = [(ni, ms) for ni in range(N_CH) for ms in range(M_SUB)]  # 12

        for mo in range(M_TILES):
            if mo == 0:
                xsb = xsb0
            else:
                xsb = xpool.tile([128, KB, M_TILE], mybir.dt.bfloat16, name="xsb")
                half = KB // 2
                nc.sync.dma_start(xsb[:, :half], xT3[:, :half, _mslice(mo)])
                nc.sync.dma_start(xsb[:, half:], xT3[:, half:, _mslice(mo)])

            if mo == 0:
                # kb-major waves (8 chains, then 4) so TensorE consumes each
                # k-block as its dequant completes instead of stalling on the
                # full weight pipeline.
                for wave in (chains[:8], chains[8:]):
                    pts = {}
                    for c in wave:
                        pts[c] = ppool.tile([128, N_FREE], mybir.dt.float32, name="pt")
                    for kb in range(KB):
                        for ni, ms in wave:
                            nc.tensor.matmul(
                                pts[(ni, ms)][:],
                                xsb[:, kb, ms * 128:(ms + 1) * 128],
                                wsb[:, kb, ni * N_FREE:(ni + 1) * N_FREE],
                                start=(kb == 0),
                                stop=(kb == KB - 1),
                            )
                    for ni, ms in wave:
                        evict(pts[(ni, ms)], mo, ms, ni)
            else:
                # Steady state: interleave the 3 n-chunks per m-subtile so
                # consecutive matmuls share the stationary operand.
                for ms in range(M_SUB):
                    pts = [
                        ppool.tile([128, N_FREE], mybir.dt.float32, name="pt")
                        for _ in range(N_CH)
                    ]
                    for kb in range(KB):
                        for ni in range(N_CH):
                            nc.tensor.matmul(
                                pts[ni][:],
                                xsb[:, kb, ms * 128:(ms + 1) * 128],
                                wsb[:, kb, ni * N_FREE:(ni + 1) * N_FREE],
                                start=(kb == 0),
                                stop=(kb == KB - 1),
                            )
                    for ni in range(N_CH):
                        evict(pts[ni], mo, ms, ni)

    nc.compile()
    _nc_cache.append(nc)
    return nc


def _prep_inputs(x, weight, scale):
    """Host-side layout prep + sharding. Returns per-core in_maps."""
    xT = np.ascontiguousarray(
        x.reshape(M, K).astype(BF16).T
    )  # [K, M] bf16, replicated to all cores
    in_maps = []
    for c in range(NCORES):
        w_c = weight[c * N:(c + 1) * N, :]           # [N, K] f32
        wT_c = np.ascontiguousarray(w_c.astype(BF16).T)  # [K, N] bf16
        s_c = scale[c * NB:(c + 1) * NB, :]          # [NB, KB] f32
        sc_c = np.ascontiguousarray(
            np.broadcast_to(s_c.T[None, :, :], (128, KB, NB))
        ).astype(np.float32)                         # [128, KB, NB]
        in_maps.append({"xT": xT, "wT": wT_c, "sc": sc_c})
    return in_maps


def run(x, weight, scale, **spmd_kwargs):
    """Build, run on 8 cores, gather. Returns (y_full, BassKernelResults)."""
    nc = _build_nc()
    in_maps = _prep_inputs(x, weight, scale)
    res = run_bass_kernel_spmd(nc, in_maps, core_ids=list(range(NCORES)), **spmd_kwargs)
    y = np.concatenate([r["y"] for r in res.results], axis=1)  # [M, OUT]
    return y.reshape(B, S, OUT).astype(np.float32), res


def kernel(x, weight, scale):
    y, _ = run(np.asarray(x), np.asarray(weight), np.asarray(scale))
    return y
